# revision 27
# baseline (speedup 1.0000x reference)
"""Trainium2 Bass kernel for nn_Block_88476326297957.

CLIP-style attention-pooling transformer block:
  - 128 cls queries attend over 196*128 = 25088 key/value tokens
  - layernorm -> Q/K/V projections (768x768) -> softmax(QK^T/8) attention
    (the predictor gate reduces to exactly 0.5*attn since softmax over a
    singleton axis is identically 1) -> residual -> LN -> MLP -> residual.

Key algebraic restructuring vs a direct lowering:
  - LN mean-centering is linear, so it folds into the weights host-side:
    (x - mu 1^T) @ W^T == x @ (C W^T) with C = I - 11^T/768, i.e. just
    column-centered weights. The per-token 1/sigma scale is applied via
    the activation engine's per-partition `scale` operand at exp time
    (for scores) and at V-evacuation time (for values).
  - Q never materializes on device: U = centered(Wk^T) @ q  ([768, H*128],
    host-computed, fp8) turns the whole scores computation into ONE fused
    x @ U matmul per token tile - no K projection, no K evacuation, no
    transposes anywhere in the main loop (x is shipped pre-transposed).
  - Per-token r = 1/sigma is computed host-side from the same fp8-quantized
    x the device consumes, shipped as two pre-scaled [128, nt, 4] tables
    (0.125*r/WS for exp, r/WS for V).

Sharding: first 24576 kv tokens split 3072/core across 8 cores; final 512
tokens computed redundantly everywhere so the single [65,1536] bf16
AllReduce of the sharded partials hides under the tail compute. Phase 3
(residual + LN + MLP on the 128 queries) is replicated; core 0's output
is returned.

PSUM budget (8 banks): ctx 3 + {sc0,sc1,sc2,Va,Vb} rotating pool 5.
"""

import sys
import types

import numpy as np
import ml_dtypes

# ---------------------------------------------------------------------------
# Problem constants (hardcoded per the harness contract)
# ---------------------------------------------------------------------------
DIM = 768
HEADS = 12
HD = 64
L = 196
N = 128
NCORES = 8
TOKENS = L * N              # 25088 kv tokens
TAIL = 512                  # tokens computed redundantly on every core
SHARD = (TOKENS - TAIL) // NCORES   # 3072 sharded tokens per core
TPC = SHARD + TAIL          # 3584 tokens processed per core (7 x 512)
EPS = 1e-5
WS_U = 16.0                 # fp8 pre-scale on U (fused Wk^T q)
WS_V = 16.0                 # fp8 pre-scale on Wv
ICH = DIM // 128            # 6 contraction chunks of 128
HQ = HEADS * 128            # 1536 score columns (head-major)
N_WARM = 2                  # keep-warm AllReduce chain length


def _ensure_ntff_hook():
    """Register the axon NTFF profiling hook if the image's antenv lacks it."""
    if "antenv.axon_hooks" in sys.modules:
        return
    mod = types.ModuleType("antenv.axon_hooks")
    _hook = [None]
    mod.set_axon_ntff_profile_hook = lambda h: _hook.__setitem__(0, h)
    mod.get_axon_ntff_profile_hook = lambda: _hook[0]
    sys.modules["antenv.axon_hooks"] = mod
    try:
        import antenv

        antenv.axon_hooks = mod
        from trn_agent_boot.trn_boot import _ntff_profile_via_ctypes

        mod.set_axon_ntff_profile_hook(
            _ntff_profile_via_ctypes("/opt/axon/libaxon_pjrt.so")
        )
    except Exception:
        pass


def build(tpc=TPC):
    """Build the Bass module (one program, run SPMD on 8 cores)."""
    import concourse.tile as tile
    from concourse import bacc, mybir
    from concourse.masks import make_identity

    f32 = mybir.dt.float32
    f32r = mybir.dt.float32r
    bf16 = mybir.dt.bfloat16
    fp8 = mybir.dt.float8e4

    nc = bacc.Bacc("TRN2", target_bir_lowering=False, debug=False,
                   num_devices=NCORES)

    nt = tpc // 512
    assert tpc == nt * 512, "tpc must be a multiple of 512"
    # xT6[tile, p, ic, t]: x token (tile*512 + t), feature (ic*128 + p), fp8.
    # Fully transposed on host so every matmul consumes it directly.
    xT6 = nc.declare_dram_parameter("xT6", [nt, 128, ICH, 512], fp8,
                                    isOutput=False)
    # r tables [p, tile, s] (token = tile*512 + s*128 + p), pre-scaled.
    rex_d = nc.declare_dram_parameter("rex", [128, nt, 4], f32, isOutput=False)
    rv_d = nc.declare_dram_parameter("rv", [128, nt, 4], f32, isOutput=False)
    q0_d = nc.declare_dram_parameter("q0", [N, DIM], f32, isOutput=False)
    # U8[gpair, p, 2, hq]: fused scores weight (centered Wk^T q * WS_U), fp8,
    # g-chunked so the first matmul can start after 1/3 of the load
    u_d = nc.declare_dram_parameter("u8", [ICH // 2, 128, 2, HQ], fp8,
                                    isOutput=False)
    # wv8[gpair, p, 2, o]: centered Wv^T * WS_V, fp8
    wv_d = nc.declare_dram_parameter("wv8", [ICH // 2, 128, 2, DIM], fp8,
                                     isOutput=False)
    # [w(fc,proj), p, ichunk, o] bf16, g2 folded into fc
    mlp_d = nc.declare_dram_parameter("mlpT", [2, 128, ICH, DIM], bf16,
                                      isOutput=False)
    mlpb_d = nc.declare_dram_parameter("mlp_b", [2, DIM], f32r, isOutput=False)
    out_d = nc.declare_dram_parameter("out", [N, DIM], f32, isOutput=True)
    import os as _os
    _dbg = bool(_os.environ.get("KERNEL_DEBUG"))
    if _dbg:
        dbg_ctx = nc.declare_dram_parameter("dbg_ctx",
                                            [128, HEADS * (HD + 1)], bf16,
                                            isOutput=True)
        dbg_q1 = nc.declare_dram_parameter("dbg_q1", [N, DIM], f32,
                                           isOutput=True)

    n_tiles_a = nt - 1          # sharded tiles (AllReduced)
    chunks = [list(range(n_tiles_a)), [n_tiles_a]]
    rg = [list(range(NCORES))]

    with tile.TileContext(nc) as tc:
        with (
            tc.tile_pool(name="singles", bufs=1) as singles,
            tc.tile_pool(name="dram", bufs=2, space="DRAM") as dram,
        ):
            # ---- resident weights & constants -------------------------------
            ident_bf = singles.tile([128, 128], bf16, tag="ident_bf")
            make_identity(nc, ident_bf)
            eps_sb = singles.tile([128, 1], f32, tag="eps")
            nc.vector.memset(eps_sb, EPS)
            ones1f = singles.tile([1, 128], f32, tag="ones1f")
            nc.vector.memset(ones1f, 1.0)
            ones1 = singles.tile([1, 128], f32r, tag="ones1")
            nc.vector.tensor_copy(out=ones1[:, :], in_=ones1f[:, :])
            # warm the EXP act table before the first real exp needs it
            junk0 = singles.tile([1, 1], f32, tag="junk0")
            nc.scalar.activation(out=junk0[:, :], in_=eps_sb[0:1, 0:1],
                                 func=mybir.ActivationFunctionType.Exp,
                                 scale=1.0)

            u8 = singles.tile([128, ICH, HQ], fp8, tag="u8")
            wv = singles.tile([128, ICH, DIM], fp8, tag="wv")
            for g in range(ICH // 2):
                nc.gpsimd.dma_start(out=u8[:, 2 * g:2 * g + 2, :],
                                    in_=u_d[g, :, :, :])
                nc.scalar.dma_start(out=wv[:, 2 * g:2 * g + 2, :],
                                    in_=wv_d[g, :, :, :])
            rex = singles.tile([128, nt, 4], f32, tag="rex")
            rv = singles.tile([128, nt, 4], f32, tag="rv")
            nc.sync.dma_start(out=rex[:, :, :], in_=rex_d[:, :, :])
            nc.sync.dma_start(out=rv[:, :, :], in_=rv_d[:, :, :])

            wfc = singles.tile([128, ICH, DIM], bf16, tag="wfc")
            wpj = singles.tile([128, ICH, DIM], bf16, tag="wpj")
            fcb = singles.tile([1, DIM], f32r, tag="fcb")
            pjb = singles.tile([1, DIM], f32r, tag="pjb")

            def load_mlp_weights():
                # on the sync queue, emitted after the main-loop x DMAs so
                # these 2.4MB don't compete with u8/wv/x during the ramp
                for w_t, wi in ((wfc, 0), (wpj, 1)):
                    nc.sync.dma_start(out=w_t[:, :, :],
                                      in_=mlp_d[wi, :, :, :])
                nc.sync.dma_start(out=fcb[:, :], in_=mlpb_d[0:1, :])
                nc.sync.dma_start(out=pjb[:, :], in_=mlpb_d[1:2, :])

            # Free-running keep-warm AllReduce chain (reduces uninitialized
            # DRAM, results unused): absorbs launch stagger and keeps the
            # collective firmware hot so the real AllReduce starts fast.
            cc_d_in = [dram.tile([1, 128], bf16, tag=f"cc_d_in{k}",
                                 name=f"cc_d_in{k}") for k in range(N_WARM)]
            cc_d_out = [dram.tile([1, 128], bf16, tag=f"cc_d_out{k}",
                                  name=f"cc_d_out{k}", addr_space="Shared")
                        for k in range(N_WARM)]
            for k in range(N_WARM):
                nc.gpsimd.collective_compute(
                    "AllReduce", mybir.AluOpType.add,
                    replica_groups=rg,
                    ins=[cc_d_in[k].opt()], outs=[cc_d_out[k].opt()])

            # persistent across phase 2+3 (scalar queue: q0 is phase-3 only)
            q0 = singles.tile([N, DIM], f32, tag="q0")
            nc.scalar.dma_start(out=q0[:, :], in_=q0_d[:, :])
            # ctx accumulates TRANSPOSED: [q, head, hd+1] (flat 780 cols)
            HP = HD + 1
            ctx_sbA = singles.tile([128, HEADS * HP], bf16, tag="ctx_sbA")
            ctx_sbB = singles.tile([128, HEADS * HP], bf16, tag="ctx_sbB")

            with (
                tc.tile_pool(name="ctxps", bufs=2, space="PSUM") as ctxps,
                tc.tile_pool(name="ps", bufs=6, space="PSUM") as ps,
                tc.tile_pool(name="xt", bufs=3) as xtp,
                tc.tile_pool(name="vt", bufs=3) as vtp,
                tc.tile_pool(name="expp", bufs=3) as expp,
            ):
                # heads 0-6 in bank A (7*65=455 f32), heads 7-11 in bank B
                ctx_psA = ctxps.tile([128, 512], f32, tag="ctx", name="ctxA")
                ctx_psB = ctxps.tile([128, 512], f32, tag="ctx", name="ctxB")
                # AllReduce in 2 head-halves so the combine pipelines under
                # the second half's ring time
                cc_in = [dram.tile([128, 6 * HP], bf16, tag=f"cc_in{k}",
                                   name=f"cc_in{k}") for k in range(2)]
                cc_out = [dram.tile([128, 6 * HP], bf16, tag=f"cc_out{k}",
                                    name=f"cc_out{k}", addr_space="Shared")
                          for k in range(2)]
                for ci, chunk in enumerate(chunks):
                    n_pairs_chunk = len(chunk) * 2
                    pair_idx = 0
                    for ti in chunk:
                        x_t = xtp.tile([128, ICH, 512], fp8, tag="x")
                        if ti < 2:
                            # split first loads so compute starts sooner
                            for gg in range(ICH // 2):
                                nc.sync.dma_start(
                                    out=x_t[:, 2 * gg:2 * gg + 2, :],
                                    in_=xT6[ti, :, 2 * gg:2 * gg + 2, :])
                        else:
                            nc.sync.dma_start(out=x_t[:, :, :],
                                              in_=xT6[ti, :, :, :])
                        v_sb = vtp.tile([128, 4, HEADS, HD + 4], fp8, tag="v")
                        nc.vector.memset(v_sb[:, :, :, HD:HD + 1], 1.0)
                        for sp in range(0, 4, 2):
                            e8 = expp.tile([128, 2, HEADS, 128], fp8, tag="e")
                            for s in range(sp, sp + 2):
                                ssl = slice(s * 128, (s + 1) * 128)
                                # scores thirds + V (512/256 split), all
                                # sharing the stationary xT slice per g
                                scs = [ps.tile([128, 512], f32, tag="big",
                                               name=f"sc{j}")
                                       for j in range(3)]
                                va = ps.tile([128, 512], f32, tag="big",
                                             name="va")
                                vb = ps.tile([128, 512], f32, tag="big",
                                             name="vb")
                                for g in range(ICH // 2):
                                    st = (g == 0)
                                    sp_ = (g == ICH // 2 - 1)
                                    lhs = x_t[:, 2 * g:2 * g + 2, ssl]
                                    # leader loads the stationary xT slice;
                                    # followers reuse the PE-resident weights
                                    nc.tensor.matmul(
                                        va[:, :], lhsT=lhs,
                                        rhs=wv[:, 2 * g:2 * g + 2, 0:512],
                                        perf_mode=mybir.MatmulPerfMode.DoubleRow,
                                        start=st, stop=sp_)
                                    nc.tensor.matmul(
                                        vb[:, 0:256], lhsT=lhs,
                                        rhs=wv[:, 2 * g:2 * g + 2, 512:768],
                                        perf_mode=mybir.MatmulPerfMode.DoubleRow,
                                        start=st, stop=sp_)
                                    for j in range(3):
                                        nc.tensor.matmul(
                                            scs[j][:, :], lhsT=lhs,
                                            rhs=u8[:, 2 * g:2 * g + 2,
                                                   j * 512:(j + 1) * 512],
                                            perf_mode=mybir.MatmulPerfMode.DoubleRow,
                                            start=st, stop=sp_)
                                # V evacuation with per-token r/WS_V scale
                                nc.vector.tensor_scalar_mul(
                                    out=v_sb[:, s, 0:8, 0:HD],
                                    in0=va[:, :].rearrange(
                                        "p (h d) -> p h d", h=8),
                                    scalar1=rv[:, ti, s:s + 1])
                                nc.vector.tensor_scalar_mul(
                                    out=v_sb[:, s, 8:12, 0:HD],
                                    in0=vb[:, 0:256].rearrange(
                                        "p (h d) -> p h d", h=4),
                                    scalar1=rv[:, ti, s:s + 1])
                                # exp with per-token 0.125*r/WS_U scale
                                for j in range(3):
                                    nc.scalar.activation(
                                        out=e8[:, s - sp, 4 * j:4 * j + 4, :],
                                        in_=scs[j][:, :].rearrange(
                                            "p (h q) -> p h q", h=4),
                                        func=mybir.ActivationFunctionType.Exp,
                                        scale=rex[:, ti, s:s + 1])
                            first = pair_idx == 0
                            last = pair_idx == n_pairs_chunk - 1
                            for h in range(HEADS):
                                # ctxT[q, h, hd+1]: e8 stationary, v moving.
                                # start=True resets the WHOLE psum bank, so
                                # h==0 (bank A) and h==7 (bank B) lead.
                                if h < 7:
                                    dst = ctx_psA[:, h * HP:(h + 1) * HP]
                                else:
                                    dst = ctx_psB[:, (h - 7) * HP:(h - 6) * HP]
                                nc.tensor.matmul(
                                    dst,
                                    lhsT=e8[:, :, h, :],
                                    rhs=v_sb[:, sp:sp + 2, h, 0:HP],
                                    perf_mode=mybir.MatmulPerfMode.DoubleRow,
                                    start=(first and h in (0, 7)), stop=last,
                                    skip_group_check=True)
                            pair_idx += 1

                    # ---- end of chunk: evacuate partials; the sharded
                    # chunk's partials AllReduce under the tail's compute
                    ctx_sb = ctx_sbA if ci == 0 else ctx_sbB
                    nc.vector.tensor_copy(out=ctx_sb[:, 0:7 * HP],
                                          in_=ctx_psA[:, 0:7 * HP])
                    nc.vector.tensor_copy(out=ctx_sb[:, 7 * HP:HEADS * HP],
                                          in_=ctx_psB[:, 0:5 * HP])
                    if ci == 0:
                        for k in range(2):
                            nc.sync.dma_start(
                                out=cc_in[k][:, :],
                                in_=ctx_sb[:, k * 6 * HP:(k + 1) * 6 * HP])
                            nc.gpsimd.collective_compute(
                                "AllReduce", mybir.AluOpType.add,
                                replica_groups=rg,
                                ins=[cc_in[k].opt()], outs=[cc_out[k].opt()])
                        load_mlp_weights()

                if _dbg:
                    nc.sync.dma_start(out=dbg_ctx[:, :],
                                      in_=ctx_sbA[:, 0:HQ])

            # ---- phase 3: combine + MLP (replicated on all cores) -----------
            with (
                tc.tile_pool(name="fin", bufs=1) as fin,
                tc.tile_pool(name="stats3", bufs=4) as stats3,
                tc.tile_pool(name="ps3", bufs=2, space="PSUM") as ps3,
                tc.tile_pool(name="ps3r", bufs=2, space="PSUM") as ps3r,
            ):
                # warm the Sqrt/Sigmoid act tables during the AllReduce wait
                # (queue order places these after the last main-loop exp)
                junk1 = fin.tile([1, 2], f32, tag="junk1")
                nc.scalar.activation(out=junk1[:, 0:1], in_=eps_sb[0:1, 0:1],
                                     func=mybir.ActivationFunctionType.Sqrt,
                                     scale=1.0)
                nc.scalar.activation(out=junk1[:, 1:2], in_=eps_sb[0:1, 0:1],
                                     func=mybir.ActivationFunctionType.Sigmoid,
                                     scale=1.0)
                # combine reduced shard partials with the local tail partial:
                # ctxT is already [q, h, hd+1] so no transposes are needed;
                # pipelined per AllReduce half (6 heads = 384 ctx columns)
                HP = HD + 1
                redA = fin.tile([128, HEADS * HP], bf16, tag="redA")
                red = fin.tile([128, HEADS * HP], bf16, tag="red")
                ctxf = fin.tile([N, DIM], f32, tag="ctxf")
                rcp = fin.tile([128, HEADS, 1], f32, tag="rcp")
                q1 = fin.tile([N, DIM], f32, tag="q1")
                st4 = fin.tile([128, 2, 6], f32, tag="st4")
                dma_q = [nc.sync, nc.scalar]
                for k in range(2):
                    ksl = slice(k * 6 * HP, (k + 1) * 6 * HP)
                    dma_q[k].dma_start(out=redA[:, ksl], in_=cc_out[k][:, :])
                for k in range(2):
                    ksl = slice(k * 6 * HP, (k + 1) * 6 * HP)
                    hsl = slice(k * 6, (k + 1) * 6)
                    nc.vector.tensor_add(out=red[:, ksl], in0=redA[:, ksl],
                                         in1=ctx_sbB[:, ksl])
                    redv = red[:, ksl].rearrange("p (h d) -> p h d", h=6)
                    # rcp = 0.5 / denom, broadcast multiply, then residual
                    # add and LN partial stats for this 384-col half
                    nc.vector.tensor_scalar_mul(out=rcp[:, hsl, 0],
                                                in0=redv[:, :, HD],
                                                scalar1=2.0)
                    nc.vector.reciprocal(out=rcp[:, hsl, :],
                                         in_=rcp[:, hsl, :])
                    csl = slice(k * 6 * HD, (k + 1) * 6 * HD)
                    nc.vector.tensor_mul(
                        out=ctxf[:, csl].rearrange("p (h d) -> p h d", h=6),
                        in0=redv[:, :, 0:HD],
                        in1=rcp[:, hsl, :].broadcast_to((128, 6, HD)))
                    nc.vector.tensor_add(out=q1[:, csl], in0=q0[:, csl],
                                         in1=ctxf[:, csl])
                    nc.vector.bn_stats(out=st4[:N, k, :], in_=q1[:, csl])
                if _dbg:
                    nc.sync.dma_start(out=dbg_q1[:, :], in_=q1[:, :])
                # LN(q1) -> h (aggregate the 2 partial stats)
                mv3 = stats3.tile([128, 2], f32, tag="mv")
                nc.vector.bn_aggr(out=mv3[:N, :], in_=st4[:N, :, :])
                sd3 = stats3.tile([128, 1], f32, tag="sd")
                nc.scalar.activation(out=sd3[:N], in_=mv3[:N, 1:2],
                                     func=mybir.ActivationFunctionType.Sqrt,
                                     bias=eps_sb[:N], scale=1.0)
                r3 = stats3.tile([128, 1], f32, tag="r")
                nc.vector.reciprocal(out=r3[:N], in_=sd3[:N])
                nmr3 = stats3.tile([128, 1], f32, tag="nmr")
                nc.vector.tensor_scalar(out=nmr3[:N], in0=mv3[:N, 0:1],
                                        scalar1=r3[:N], scalar2=-1.0,
                                        op0=mybir.AluOpType.mult,
                                        op1=mybir.AluOpType.mult)
                # LN apply on the vector engine: h = q1*r + (-mu*r)
                h_sb = fin.tile([N, DIM], bf16, tag="h")
                nc.vector.tensor_scalar(out=h_sb[:, :], in0=q1[:, :],
                                        scalar1=r3[:N], scalar2=nmr3[:N],
                                        op0=mybir.AluOpType.mult,
                                        op1=mybir.AluOpType.add)

                def transpose6_bf(src, pool, tag):
                    dst = pool.tile([128, ICH, 128], bf16, tag=tag, name=tag)
                    for ic in range(ICH):
                        tp = ps3r.tile([128, 512], bf16, tag="big3r")
                        nc.tensor.transpose(tp[:, 0:128],
                                            src[:, ic * 128:(ic + 1) * 128],
                                            ident_bf[:, :])
                        nc.vector.tensor_copy(out=dst[:, ic, :],
                                              in_=tp[:, 0:128])
                    return dst

                def mlp_layer(inpT, w_t, bias_row):
                    """out[t, o] = inpT.T @ w + bias ; returns psum tiles.

                    The two 384-col halves interleave so each lhsT loads once
                    (the second half's matmul reuses the PE-resident weights).
                    """
                    outs = [ps3.tile([128, 512], f32, tag="big3",
                                     name=f"mlp{half}") for half in range(2)]
                    for step in range(ICH + 1):
                        for half in range(2):
                            osl = slice(half * 384, (half + 1) * 384)
                            if step == 0:
                                nc.tensor.matmul(
                                    outs[half][:, 0:384],
                                    lhsT=ones1[0:1, :],
                                    rhs=bias_row[:, osl],
                                    start=True, stop=False)
                            else:
                                ic = step - 1
                                nc.tensor.matmul(
                                    outs[half][:, 0:384],
                                    lhsT=inpT[:, ic, :],
                                    rhs=w_t[:, ic, osl],
                                    start=False, stop=(ic == ICH - 1))
                    return outs

                hT = transpose6_bf(h_sb, fin, "hT")
                m1ps = mlp_layer(hT, wfc, fcb)
                m1 = fin.tile([N, DIM], f32, tag="m1")
                sig = fin.tile([N, DIM], f32, tag="sig")
                for half in range(2):
                    osl = slice(half * 384, (half + 1) * 384)
                    nc.vector.tensor_copy(out=m1[:, osl],
                                          in_=m1ps[half][:, 0:384])
                    nc.scalar.activation(out=sig[:, osl],
                                         in_=m1ps[half][:, 0:384],
                                         func=mybir.ActivationFunctionType.Sigmoid,
                                         scale=1.702)
                m2 = fin.tile([N, DIM], bf16, tag="m2")
                nc.vector.tensor_mul(out=m2[:, :], in0=m1[:, :], in1=sig[:, :])
                m2T = transpose6_bf(m2, fin, "m2T")
                m3ps = mlp_layer(m2T, wpj, pjb)
                out_sb = fin.tile([N, DIM], f32, tag="out")
                for half in range(2):
                    osl = slice(half * 384, (half + 1) * 384)
                    nc.vector.tensor_add(out=out_sb[:, osl], in0=q1[:, osl],
                                         in1=m3ps[half][:, 0:384])
                nc.sync.dma_start(out=out_d[:, :], in_=out_sb[:, :])

    nc.compile()
    n_removed = _dedup_ldweights(nc, mybir)
    assert n_removed > 100, f"ldweights dedup removed only {n_removed}"
    return nc


def _dedup_ldweights(nc, mybir):
    """Drop InstLdweights that reload the identical PE-resident weights.

    The compile pipeline splits every matmul into LDWEIGHTS + MATMULT even
    when consecutive matmuls share the same stationary tensor. Weights
    persist in the PE array across matmuls, so a reload of the same
    physical AP (same perf mode / tile position) is pure overhead. Any
    semaphore waits/updates on a dropped LDWEIGHTS move to its paired
    matmult, which sits at the same point in the PE queue.
    """
    removed = 0
    for f in nc.m.functions:
        for bb in f.blocks:
            insts = bb.instructions
            last_sig = None
            to_del = []
            for idx in range(len(insts)):
                i = insts[idx]
                tn = type(i).__name__
                if tn == 'InstMatmult':
                    # self-loading matmuls (f32/f32r) clobber the resident
                    # weights without an InstLdweights; invalidate tracking
                    # whenever a matmul's weights don't match what's loaded
                    if last_sig is None or str(i.ins[1]) != last_sig[0]:
                        last_sig = None
                    continue
                if tn != 'InstLdweights':
                    continue
                sig = (str(i.ins[0]), str(i.perf_mode), str(i.tile_position),
                       str(i.tile_size), str(i.is_transpose))
                nxt = insts[idx + 1] if idx + 1 < len(insts) else None
                si = i.sync_info
                # only drop sync-free reloads: moving waits onto the matmult
                # can exceed the ISA's sync-command slots
                if (sig != last_sig or nxt is None
                        or type(nxt).__name__ != 'InstMatmult'
                        or str(nxt.ins[1]) != sig[0]
                        or (si is not None
                            and (len(si.on_wait) or len(si.on_update)))):
                    last_sig = sig
                    continue
                to_del.append(idx)
            for idx in reversed(to_del):
                del bb.instructions[idx]
            removed += len(to_del)
    return removed


_BUILD_CACHE = {}


def _get_nc(tpc=TPC):
    if tpc not in _BUILD_CACHE:
        _BUILD_CACHE[tpc] = build(tpc)
    return _BUILD_CACHE[tpc]


def prep_inputs(x, cls, g1, b1, g2, b2, Wq, Wk, Wv, fc_w, fc_b, proj_w,
                proj_b, tpc=TPC):
    """Host-side sharding + weight prep. Returns per-core input maps."""
    x = np.asarray(x, np.float32)
    cls = np.asarray(cls, np.float32)
    g1 = np.asarray(g1, np.float32)
    b1 = np.asarray(b1, np.float32)
    g2 = np.asarray(g2, np.float32)
    b2 = np.asarray(b2, np.float32)
    assert np.allclose(b1, 0.0), "nonzero b1 not supported by this build"
    if not np.allclose(g1, 1.0):
        raise NotImplementedError("non-unit g1")
    xs = x.reshape(L * N, DIM)
    cls2 = cls.reshape(N, DIM)

    # fp8-quantize x once; all device math and host r-stats use it
    xq8 = xs.astype(ml_dtypes.float8_e4m3)
    xqf = xq8.astype(np.float32)
    var = xqf.var(axis=1)
    r = 1.0 / np.sqrt(var + EPS)                       # [tokens]

    # host phase 1: q = LN(cls) @ Wq.T, then fuse U = centered(Wk^T) q
    mu = cls2.mean(axis=1, keepdims=True)
    cvar = cls2.var(axis=1)
    q0h = (cls2 - mu) / np.sqrt(cvar + EPS)[:, None]
    qh = q0h @ np.asarray(Wq, np.float32).T            # [N, DIM]
    Wk3 = np.asarray(Wk, np.float32).reshape(HEADS, HD, DIM)
    qh3 = qh.reshape(N, HEADS, HD)
    U = np.einsum("hdD,qhd->Dhq", Wk3, qh3).reshape(DIM, HQ)
    U -= U.mean(axis=0, keepdims=True)                 # fold LN centering
    WvT = np.ascontiguousarray(np.asarray(Wv, np.float32).T)
    WvT = WvT - WvT.mean(axis=0, keepdims=True)        # fold LN centering

    def chunk_major(wT):
        # [DIM, cols] -> [p(128), ichunk, cols]
        cols = wT.shape[1]
        return np.ascontiguousarray(
            wT.reshape(ICH, 128, cols).transpose(1, 0, 2))

    def g_chunked(cm):
        # [128, ICH, cols] -> [ICH//2, 128, 2, cols]
        cols = cm.shape[2]
        return np.ascontiguousarray(
            cm.reshape(128, ICH // 2, 2, cols).transpose(1, 0, 2, 3))

    u8 = g_chunked(chunk_major((U * WS_U).astype(ml_dtypes.float8_e4m3)))
    wv8 = g_chunked(chunk_major((WvT * WS_V).astype(ml_dtypes.float8_e4m3)))

    def foldT(w, g):
        return np.ascontiguousarray((np.asarray(w, np.float32) * g[None, :]).T)

    mlpT = np.stack([
        chunk_major(foldT(fc_w, g2)),
        chunk_major(np.ascontiguousarray(np.asarray(proj_w, np.float32).T)),
    ]).astype(ml_dtypes.bfloat16)
    fc_b_eff = np.asarray(fc_b, np.float32) + np.asarray(fc_w, np.float32) @ b2
    mlp_b = np.stack([fc_b_eff, np.asarray(proj_b, np.float32)])

    nt = tpc // 512
    tail8 = xq8[NCORES * SHARD:]
    r_tail = r[NCORES * SHARD:]
    in_maps = []
    for c in range(NCORES):
        shard8 = np.concatenate([xq8[c * SHARD:(c + 1) * SHARD], tail8])
        # [tokens, DIM] -> [tile, p(feature%128), ic, t(512)]
        xT = np.ascontiguousarray(
            shard8.reshape(nt, 512, ICH, 128).transpose(0, 3, 2, 1))
        rc = np.concatenate([r[c * SHARD:(c + 1) * SHARD], r_tail])
        # token = tile*512 + s*128 + p  ->  [p, tile, s]
        rts = np.ascontiguousarray(
            rc.reshape(nt, 4, 128).transpose(2, 0, 1))
        in_maps.append({
            "xT6": xT,
            "rex": rts * (0.125 / WS_U),
            "rv": rts * (1.0 / WS_V),
            "q0": q0h,
            "u8": u8,
            "wv8": wv8,
            "mlpT": mlpT,
            "mlp_b": mlp_b,
        })
    return in_maps


def run(inputs, tpc=TPC, trace=False, trace_cores=None):
    _ensure_ntff_hook()
    from concourse.bass_utils import run_bass_kernel_spmd

    nc = _get_nc(tpc)
    in_maps = prep_inputs(
        inputs["x"], inputs["cls"], inputs["g1"], inputs["b1"], inputs["g2"],
        inputs["b2"], inputs["Wq"], inputs["Wk"], inputs["Wv"],
        inputs["fc_w"], inputs["fc_b"], inputs["proj_w"], inputs["proj_b"],
        tpc=tpc)
    res = run_bass_kernel_spmd(nc, in_maps, core_ids=list(range(NCORES)),
                               trace=trace, trace_cores=trace_cores)
    out = np.asarray(res.results[0]["out"], np.float32).reshape(1, N, DIM)
    return out, res


def kernel(**inputs):
    out, _ = run(inputs, tpc=TPC, trace=False)
    return out
